# revision 1
# baseline (speedup 1.0000x reference)
"""Binary residual block (sign-conv x3) on 8 TRN2 NeuronCores.

Data-parallel: batch 64 is split 8 ways (8 images per core); binarized
weights are replicated. Per core the three convs are computed as PE
matmuls with channels on the partition (contraction) dim:

  conv1 3x3/s2 + shortcut 1x1/s2: x is split into two fp16 limbs
    (hi = fp16(x), lo = fp16(x - hi)); +-1 weights are exact in fp16, so
    accumulating both limb matmuls in fp32 PSUM reproduces fp32 accuracy
    at bf16 PE throughput.
  conv2 3x3/s1: inputs are sign() outputs (+-1/0) -> exact in fp16.

Each conv output quarter ([kout-tile, half of 28 rows] = 392 positions)
is one PSUM accumulation group over taps/limbs; Sign is applied by the
scalar engine straight out of PSUM.
"""

import numpy as np

P = 128
H = W = 56
OH = OW = 28
HP = 58         # zero-padded input edge (56 + 2)
H2P = 30        # zero-padded sign1 edge (28 + 2)
N_CORES = 8
IMG = 8         # images per core

_CACHE = {}


def _build(n_cores=N_CORES, img=IMG, repeat=1):
    import concourse.bass as bass  # noqa: F401
    import concourse.tile as tile
    from concourse import bacc, mybir

    AF = mybir.ActivationFunctionType
    f32 = mybir.dt.float32
    f16 = mybir.dt.float16

    nc = bacc.Bacc("TRN2", target_bir_lowering=False, debug=False,
                   num_devices=n_cores)
    x_d = nc.dram_tensor("x", [img, 128, H, W], f32, kind="ExternalInput")
    w1_d = nc.dram_tensor("w1", [256, 128, 3, 3], f32, kind="ExternalInput")
    w2_d = nc.dram_tensor("w2", [256, 256, 3, 3], f32, kind="ExternalInput")
    wsc_d = nc.dram_tensor("wsc", [256, 128, 1, 1], f32, kind="ExternalInput")
    y_d = nc.dram_tensor("y", [img, 256, OH, OW], f32, kind="ExternalOutput")

    with tile.TileContext(nc) as tc:
        with (
            tc.tile_pool(name="wpool", bufs=1) as wpool,
            tc.tile_pool(name="xin", bufs=2) as xin_pool,
            tc.tile_pool(name="xlimb", bufs=2) as xlimb_pool,
            tc.tile_pool(name="s1pool", bufs=2) as s1pool,
            tc.tile_pool(name="opool", bufs=2) as opool,
            tc.tile_pool(name="pc1", bufs=4, space="PSUM") as pc1,
            tc.tile_pool(name="pc2", bufs=4, space="PSUM") as pc2,
        ):
            # ---------- weights: gather transposed, sign -> fp16 ----------
            with tc.tile_pool(name="wstage", bufs=1) as wstage:
                w1s = wstage.tile([P, 9, 2, P], f32, tag="w1s")
                nc.sync.dma_start(
                    w1s[:],
                    w1_d[:].rearrange("(ko m) c kh kw -> c (kh kw) ko m", ko=2))
                w1t = wpool.tile([P, 9, 2, P], f16, tag="w1t")
                nc.scalar.activation(w1t[:], w1s[:], AF.Sign)

                w2s = wstage.tile([P, 2, 9, 2, P], f32, tag="w2s")
                for ct in range(2):
                    nc.sync.dma_start(
                        w2s[:, ct],
                        w2_d[:, ct * P:(ct + 1) * P].rearrange(
                            "(ko m) c kh kw -> c (kh kw) ko m", ko=2))
                w2t = wpool.tile([P, 2, 9, 2, P], f16, tag="w2t")
                nc.scalar.activation(w2t[:], w2s[:], AF.Sign)

                wscs = wstage.tile([P, 2, P], f32, tag="wscs")
                nc.sync.dma_start(
                    wscs[:],
                    wsc_d[:, :, 0, 0].rearrange("(ko m) c -> c ko m", ko=2))
                wsct = wpool.tile([P, 2, P], f16, tag="wsct")
                nc.scalar.activation(wsct[:], wscs[:], AF.Sign)

            # ---------- per-image stages ----------
            def load(i):
                x32 = xin_pool.tile([P, HP, HP], f32, tag="x32")
                nc.gpsimd.memset(x32[:], 0.0)
                nc.sync.dma_start(x32[:, 1:57, 1:57], x_d[i])
                xhi = xlimb_pool.tile([P, HP, HP], f16, tag="xhi")
                nc.vector.tensor_copy(xhi[:], x32[:])
                xlo = xlimb_pool.tile([P, HP, HP], f16, tag="xlo")
                nc.vector.tensor_sub(xlo[:], x32[:], xhi[:])
                return xhi, xlo

            def quad(limb):
                # [c, h2, w2, h, w] = xpad[c, 2h+h2, 2w+w2]
                return limb[:].rearrange(
                    "c (h h2) (w w2) -> c h2 w2 h w", h2=2, w2=2)

            def conv1(i, xhi, xlo):
                s1 = s1pool.tile([P, 2, H2P, H2P], f16, tag="s1")
                nc.gpsimd.memset(s1[:], 0.0)
                for ko in range(2):
                    for hf in range(2):
                        p1 = pc1.tile([P, 14, OW], f32, tag="p1")
                        cnt = 0
                        for limb in (xhi, xlo):
                            lr = quad(limb)
                            for kh in range(3):
                                for kw in range(3):
                                    rhs = lr[:, kh % 2, kw % 2,
                                             kh // 2 + 14 * hf:
                                             kh // 2 + 14 * hf + 14,
                                             kw // 2: kw // 2 + OW]
                                    nc.tensor.matmul(
                                        p1[:], w1t[:, kh * 3 + kw, ko, :], rhs,
                                        start=(cnt == 0), stop=(cnt == 17))
                                    cnt += 1
                        nc.scalar.activation(
                            s1[:, ko, 1 + 14 * hf: 15 + 14 * hf, 1:29],
                            p1[:], AF.Sign)
                return s1

            def conv2_out(i, s1, xhi, xlo):
                ou = opool.tile([P, 2, OH, OW], f32, tag="ou")
                for ko in range(2):
                    for hf in range(2):
                        p2 = pc2.tile([P, 14, OW], f32, tag="p2")
                        cnt = 0
                        for ct in range(2):
                            for kh in range(3):
                                for kw in range(3):
                                    rhs = s1[:, ct,
                                             kh + 14 * hf: kh + 14 * hf + 14,
                                             kw: kw + OW]
                                    nc.tensor.matmul(
                                        p2[:], w2t[:, ct, kh * 3 + kw, ko, :],
                                        rhs, start=(cnt == 0), stop=False)
                                    cnt += 1
                        for limb in (xhi, xlo):
                            lr = quad(limb)
                            rhs = lr[:, 1, 1, 14 * hf: 14 * hf + 14, 0:OW]
                            cnt += 1
                            nc.tensor.matmul(
                                p2[:], wsct[:, ko, :], rhs,
                                start=False, stop=(cnt == 20))
                        nc.scalar.activation(
                            ou[:, ko, 14 * hf: 14 * hf + 14, :], p2[:], AF.Sign)
                nc.sync.dma_start(
                    y_d[i].rearrange("(ko m) h w -> m ko h w", ko=2), ou[:])

            def whole_pass():
                prev = None
                for i in range(img):
                    xhi, xlo = load(i)
                    s1 = conv1(i, xhi, xlo)
                    if prev is not None:
                        conv2_out(*prev)
                    prev = (i, s1, xhi, xlo)
                conv2_out(*prev)

            if repeat == 1:
                whole_pass()
            else:
                with tc.For_i(0, repeat, 1):
                    whole_pass()

    nc.compile()
    return nc


def _get_nc(repeat=1):
    key = repeat
    if key not in _CACHE:
        _CACHE[key] = _build(repeat=repeat)
    return _CACHE[key]


def kernel(x, w1, w2, w_sc):
    from concourse import bass_utils

    x = np.ascontiguousarray(np.asarray(x, dtype=np.float32))
    w1 = np.ascontiguousarray(np.asarray(w1, dtype=np.float32))
    w2 = np.ascontiguousarray(np.asarray(w2, dtype=np.float32))
    w_sc = np.ascontiguousarray(np.asarray(w_sc, dtype=np.float32))

    nc = _get_nc()
    in_maps = [
        {"x": x[c * IMG:(c + 1) * IMG], "w1": w1, "w2": w2, "wsc": w_sc}
        for c in range(N_CORES)
    ]
    res = bass_utils.run_bass_kernel_spmd(
        nc, in_maps, core_ids=list(range(N_CORES)))
    y = np.concatenate([res.results[c]["y"] for c in range(N_CORES)], axis=0)
    return y


# revision 3
# speedup vs baseline: 1.3673x; 1.3673x over previous
"""Binary residual block (sign-conv x3) on 8 TRN2 NeuronCores.

Data-parallel: batch 64 is split 8 ways (8 images per core); binarized
weights are replicated. Per core the three convs are computed as PE
matmuls with channels on the partition (contraction) dim:

  conv1 3x3/s2 + shortcut 1x1/s2: x is split into two fp16 limbs
    (hi = fp16(x), lo = fp16(x - hi)); +-1 weights are exact in fp16, so
    accumulating both limb matmuls in fp32 PSUM reproduces fp32 accuracy
    at bf16 PE throughput.
  conv2 3x3/s1: inputs are sign() outputs (+-1/0) -> exact in fp16.

Each conv output quarter ([kout-tile, 14 of 28 rows] = 392 positions) is
one PSUM accumulation group over taps/limbs; Sign is applied by the
scalar engine straight out of PSUM.

Weights are pre-transposed on the host to the lhsT layout the PE wants
([C_in partition, tap, kout-tile, m]) so their DMAs are contiguous;
sign() itself runs on device. Padded limb/sign tiles are persistent with
the zero ring written once; per-image ops only touch the interior.
"""

import numpy as np

P = 128
H = W = 56
OH = OW = 28
HP = 58         # zero-padded input edge (56 + 2)
H2P = 30        # zero-padded sign1 edge (28 + 2)
N_CORES = 8
IMG = 8         # images per core

_CACHE = {}


def _build(n_cores=N_CORES, img=IMG, repeat=1):
    import concourse.bass as bass  # noqa: F401
    import concourse.tile as tile
    from concourse import bacc, mybir

    AF = mybir.ActivationFunctionType
    f32 = mybir.dt.float32
    f16 = mybir.dt.float16

    nc = bacc.Bacc("TRN2", target_bir_lowering=False, debug=False,
                   num_devices=n_cores)
    x_d = nc.dram_tensor("x", [img, 128, H, W], f32, kind="ExternalInput")
    # host-pretransposed lhsT layouts
    w1_d = nc.dram_tensor("w1", [P, 9, 2, P], f32, kind="ExternalInput")
    w2_d = nc.dram_tensor("w2", [P, 2, 9, 2, P], f32, kind="ExternalInput")
    wsc_d = nc.dram_tensor("wsc", [P, 2, P], f32, kind="ExternalInput")
    y_d = nc.dram_tensor("y", [img, 256, OH, OW], f32, kind="ExternalOutput")

    with tile.TileContext(nc) as tc:
        with (
            tc.tile_pool(name="wpool", bufs=1) as wpool,
            tc.tile_pool(name="xper", bufs=1) as xper,
            tc.tile_pool(name="xin", bufs=2) as xin_pool,
            tc.tile_pool(name="opool", bufs=2) as opool,
            tc.tile_pool(name="pc1", bufs=4, space="PSUM") as pc1,
            tc.tile_pool(name="pc2", bufs=4, space="PSUM") as pc2,
        ):
            # persistent padded tiles; ring zeroed once, interiors rewritten
            xhi = [xper.tile([P, HP, HP], f16, tag=f"xhi{j}",
                             name=f"xhi{j}") for j in range(2)]
            xlo = [xper.tile([P, HP, HP], f16, tag=f"xlo{j}",
                             name=f"xlo{j}") for j in range(2)]
            s1b = [xper.tile([P, 2, H2P, H2P], f16, tag=f"s1{j}",
                             name=f"s1{j}") for j in range(2)]
            for t in xhi + xlo + s1b:
                nc.gpsimd.memset(t[:], 0.0)

            # ---------- weights: contiguous DMA, sign -> fp16 ----------
            w1t = wpool.tile([P, 9, 2, P], f16, tag="w1t")
            w2t = wpool.tile([P, 2, 9, 2, P], f16, tag="w2t")
            wsct = wpool.tile([P, 2, P], f16, tag="wsct")
            with tc.tile_pool(name="wstage", bufs=1) as wstage:
                w1s = wstage.tile([P, 9, 2, P], f32, tag="w1s")
                nc.sync.dma_start(w1s[:], w1_d[:])
                nc.scalar.activation(w1t[:], w1s[:], AF.Sign)
                w2s = wstage.tile([P, 2, 9, 2, P], f32, tag="w2s")
                nc.sync.dma_start(w2s[:], w2_d[:])
                nc.scalar.activation(w2t[:], w2s[:], AF.Sign)
                wscs = wstage.tile([P, 2, P], f32, tag="wscs")
                nc.sync.dma_start(wscs[:], wsc_d[:])
                nc.scalar.activation(wsct[:], wscs[:], AF.Sign)

            # ---------- per-image stages ----------
            def load(i):
                x32 = xin_pool.tile([P, H, W], f32, tag="x32")
                nc.sync.dma_start(x32[:], x_d[i])
                hi, lo = xhi[i % 2], xlo[i % 2]
                nc.vector.tensor_copy(hi[:, 1:57, 1:57], x32[:])
                nc.vector.tensor_sub(lo[:, 1:57, 1:57], x32[:],
                                     hi[:, 1:57, 1:57])
                return hi, lo

            def quad(limb):
                # [c, h2, w2, h, w] = xpad[c, 2h+h2, 2w+w2]
                return limb[:].rearrange(
                    "c (h h2) (w w2) -> c h2 w2 h w", h2=2, w2=2)

            def conv1(i, hi, lo):
                s1 = s1b[i % 2]
                for ko in range(2):
                    for hf in range(2):
                        p1 = pc1.tile([P, 14, OW], f32, tag="p1")
                        cnt = 0
                        for limb in (hi, lo):
                            lr = quad(limb)
                            for kh in range(3):
                                for kw in range(3):
                                    rhs = lr[:, kh % 2, kw % 2,
                                             kh // 2 + 14 * hf:
                                             kh // 2 + 14 * hf + 14,
                                             kw // 2: kw // 2 + OW]
                                    nc.tensor.matmul(
                                        p1[:], w1t[:, kh * 3 + kw, ko, :], rhs,
                                        start=(cnt == 0), stop=(cnt == 17))
                                    cnt += 1
                        nc.scalar.activation(
                            s1[:, ko, 1 + 14 * hf: 15 + 14 * hf, 1:29],
                            p1[:], AF.Sign)
                return s1

            def conv2_out(i, s1, hi, lo):
                ou = opool.tile([P, 2, OH, OW], f32, tag="ou")
                for ko in range(2):
                    for hf in range(2):
                        p2 = pc2.tile([P, 14, OW], f32, tag="p2")
                        cnt = 0
                        for ct in range(2):
                            for kh in range(3):
                                for kw in range(3):
                                    rhs = s1[:, ct,
                                             kh + 14 * hf: kh + 14 * hf + 14,
                                             kw: kw + OW]
                                    nc.tensor.matmul(
                                        p2[:], w2t[:, ct, kh * 3 + kw, ko, :],
                                        rhs, start=(cnt == 0), stop=False)
                                    cnt += 1
                        for limb in (hi, lo):
                            lr = quad(limb)
                            rhs = lr[:, 1, 1, 14 * hf: 14 * hf + 14, 0:OW]
                            cnt += 1
                            nc.tensor.matmul(
                                p2[:], wsct[:, ko, :], rhs,
                                start=False, stop=(cnt == 20))
                        nc.scalar.activation(
                            ou[:, ko, 14 * hf: 14 * hf + 14, :], p2[:], AF.Sign)
                nc.sync.dma_start(
                    y_d[i].rearrange("(ko m) h w -> m ko h w", ko=2), ou[:])

            def whole_pass():
                prev = None
                for i in range(img):
                    hi, lo = load(i)
                    s1 = conv1(i, hi, lo)
                    if prev is not None:
                        conv2_out(*prev)
                    prev = (i, s1, hi, lo)
                conv2_out(*prev)

            if repeat == 1:
                whole_pass()
            else:
                with tc.For_i(0, repeat, 1):
                    whole_pass()

    nc.compile()
    return nc


def _get_nc(repeat=1):
    key = repeat
    if key not in _CACHE:
        _CACHE[key] = _build(repeat=repeat)
    return _CACHE[key]


def prep_weights(w1, w2, w_sc):
    """Host-side lhsT layout prep (pure transposition, no math)."""
    w1 = np.asarray(w1, dtype=np.float32)
    w2 = np.asarray(w2, dtype=np.float32)
    w_sc = np.asarray(w_sc, dtype=np.float32)
    # [c, kh*kw, ko, m] from (K=ko*128+m, c, kh, kw)
    w1t = np.ascontiguousarray(
        w1.transpose(1, 2, 3, 0).reshape(P, 9, 2, P))
    # [cp, ct, kh*kw, ko, m] from (K, C=ct*128+cp, kh, kw)
    w2t = np.ascontiguousarray(
        w2.reshape(2, P, 2, P, 3, 3)           # ko m ct cp kh kw
        .transpose(3, 2, 4, 5, 0, 1)           # cp ct kh kw ko m
        .reshape(P, 2, 9, 2, P))
    wsct = np.ascontiguousarray(
        w_sc[:, :, 0, 0].transpose(1, 0).reshape(P, 2, P))
    return w1t, w2t, wsct


def kernel(x, w1, w2, w_sc):
    from concourse import bass_utils

    x = np.ascontiguousarray(np.asarray(x, dtype=np.float32))
    w1t, w2t, wsct = prep_weights(w1, w2, w_sc)

    nc = _get_nc()
    in_maps = [
        {"x": x[c * IMG:(c + 1) * IMG], "w1": w1t, "w2": w2t, "wsc": wsct}
        for c in range(N_CORES)
    ]
    res = bass_utils.run_bass_kernel_spmd(
        nc, in_maps, core_ids=list(range(N_CORES)))
    y = np.concatenate([res.results[c]["y"] for c in range(N_CORES)], axis=0)
    return y


# revision 14
# speedup vs baseline: 2.5284x; 1.8492x over previous
"""Binary residual block (sign-conv x3) on 8 TRN2 NeuronCores.

Data-parallel: batch 64 is split 8 ways (8 images per core); binarized
weights are replicated. Per core the three convs run as PE matmuls with
input channels on the partition (contraction) dim:

  conv1 3x3/s2 + shortcut 1x1/s2: x is split into two fp16 limbs
    (hi = fp16(x), lo = fp16(x - hi)); +-1 weights are exact in fp16, so
    accumulating both limb matmuls in fp32 PSUM reproduces fp32 accuracy
    at full PE rate (fp32 matmul would run at 1/4 rate).
  conv2 3x3/s1: inputs are sign() outputs, exactly representable in
    fp8e4, so it runs as fp8 DoubleRow matmuls (256-deep contraction per
    instruction, ~1.7x the fp16 rate) with bit-exact integer results.

Layouts: x limbs live in parity-quadrant form Q[c, h2, w2, h, w] =
xpad[c, 2h+h2, 2w+w2] (29x30 per quadrant) so every stride-2 tap reads
unit-stride columns; sign1 lives zero-padded 30x30 per channel-tile with
a 912-byte tile stride (DoubleRow requires the K-pair stride % 16 == 0).
conv2 streams contiguous 420-lane runs (14 rows x 30 cols incl. pad);
the two pad lanes per row are junk and never read back. Each conv output
quarter is one PSUM accumulation group (conv2 + shortcut share a group);
Sign applies on the scalar engine straight out of PSUM.

Weights are pre-transposed on the host to the lhsT layouts the PE wants
(pure permutation; sign() itself runs on device). Padded tiles are
persistent: the zero ring is written once, per-image ops only touch the
interior.
"""

import numpy as np

P = 128
H = W = 56
OH = OW = 28
H2P = 30        # zero-padded sign1 edge (28 + 2)
QE = 29         # quadrant rows
QW = 30         # quadrant row pitch (28 valid + pad)
N_CORES = 8
IMG = 8         # images per core
NBUF = 3        # persistent tile sets (pipeline depth across images)

_CACHE = {}


def _build(n_cores=N_CORES, img=IMG, repeat=1):
    import concourse.bass as bass  # noqa: F401
    import concourse.tile as tile
    from concourse import bacc, mybir

    AF = mybir.ActivationFunctionType
    f32 = mybir.dt.float32
    f16 = mybir.dt.float16
    f8 = mybir.dt.float8e4
    DRPM = mybir.MatmulPerfMode.DoubleRow

    nc = bacc.Bacc("TRN2", target_bir_lowering=False, debug=False,
                   num_devices=n_cores)
    x_d = nc.dram_tensor("x", [img, 128, H, W], f32, kind="ExternalInput")
    # host-pretransposed lhsT layouts (see prep_weights)
    w1_d = nc.dram_tensor("w1", [P, 9, 2, P], f32, kind="ExternalInput")
    w2_d = nc.dram_tensor("w2", [P, 9, 2, 2, P], f32, kind="ExternalInput")
    wsc_d = nc.dram_tensor("wsc", [P, 2, P], f32, kind="ExternalInput")
    y_d = nc.dram_tensor("y", [img, 256, OH, OW], f32, kind="ExternalOutput")

    with tile.TileContext(nc) as tc:
        with (
            tc.tile_pool(name="wpool", bufs=1) as wpool,
            tc.tile_pool(name="xper", bufs=1) as xper,
            tc.tile_pool(name="xin", bufs=3) as xin_pool,
            tc.tile_pool(name="opool", bufs=2) as opool,
            tc.tile_pool(name="wstage", bufs=1) as wstage,
            tc.tile_pool(name="pc1", bufs=4, space="PSUM") as pc1,
            tc.tile_pool(name="pc2", bufs=4, space="PSUM") as pc2,
        ):
            # persistent parity-quadrant limb tiles and sign1 tiles;
            # zero ring written once, interiors rewritten per image
            xhi = [xper.tile([P, 2, 2, QE, QW], f16, tag=f"xhi{j}",
                             name=f"xhi{j}") for j in range(NBUF)]
            xlo = [xper.tile([P, 2, 2, QE, QW], f16, tag=f"xlo{j}",
                             name=f"xlo{j}") for j in range(NBUF)]
            s1b = [xper.tile([P, 2, 912], f8, tag=f"s1{j}",
                             name=f"s1{j}") for j in range(NBUF)]
            for t in xhi + xlo + s1b:
                nc.gpsimd.memset(t[:], 0.0)

            w1t = wpool.tile([P, 9, 2, P], f16, tag="w1t")
            w2t = wpool.tile([P, 9, 2, 2, P], f8, tag="w2t")
            wsct = wpool.tile([P, 2, P], f16, tag="wsct")

            def prep_w():
                w1s = wstage.tile([P, 9, 2, P], f32, tag="w1s")
                nc.sync.dma_start(w1s[:], w1_d[:])
                nc.scalar.activation(w1t[:], w1s[:], AF.Sign)
                w2s = wstage.tile([P, 9, 2, 2, P], f32, tag="w2s")
                nc.sync.dma_start(w2s[:], w2_d[:])
                nc.scalar.activation(w2t[:], w2s[:], AF.Sign)
                wscs = wstage.tile([P, 2, P], f32, tag="wscs")
                nc.sync.dma_start(wscs[:], wsc_d[:])
                nc.scalar.activation(wsct[:], wscs[:], AF.Sign)

            def load(i):
                hi, lo = xhi[i % NBUF], xlo[i % NBUF]
                x32 = xin_pool.tile([P, H, W], f32, tag="x32")
                nc.sync.dma_start(x32[:], x_d[i])
                xv = x32[:].rearrange(
                    "c (h h2) (w w2) -> c h2 w2 h w", h2=2, w2=2)
                for h2 in range(2):
                    for w2 in range(2):
                        dst = (slice(None), h2, w2,
                               slice(1 - h2, 29 - h2), slice(1 - w2, 29 - w2))
                        srcq = xv[:, 1 - h2, 1 - w2, 0:28, 0:28]
                        nc.vector.tensor_copy(hi[dst], srcq)
                        nc.vector.tensor_sub(lo[dst], srcq, hi[dst])
                return hi, lo

            def conv1(i, hi, lo):
                s1 = s1b[i % NBUF]
                for ko in range(2):
                    for hf in range(2):
                        p1 = pc1.tile([P, 14, OW], f32, tag="p1")
                        cnt = 0
                        for limb in (hi, lo):
                            for kh in range(3):
                                for kw in range(3):
                                    rhs = limb[:, kh % 2, kw % 2,
                                               kh // 2 + 14 * hf:
                                               kh // 2 + 14 * hf + 14,
                                               kw // 2: kw // 2 + OW]
                                    nc.tensor.matmul(
                                        p1[:], w1t[:, kh * 3 + kw, ko, :], rhs,
                                        start=(cnt == 0), stop=(cnt == 17))
                                    cnt += 1
                        s1v = s1[:, :, :900].rearrange(
                            "c t (h w) -> c t h w", h=H2P)
                        nc.scalar.activation(
                            s1v[:, ko, 1 + 14 * hf: 15 + 14 * hf, 1:29],
                            p1[:], AF.Sign)
                return s1

            def conv2_out(i, s1, hi, lo):
                ou = opool.tile([P, 2, OH, OW], f32, tag="ou")
                for ko in range(2):
                    for hf in range(2):
                        # 9 DoubleRow MMs over contiguous 420-lane runs
                        # (14 rows x 30 incl. pad cols); lanes with
                        # ow in {28, 29} are junk and never read.
                        p2 = pc2.tile([P, 420], f32, tag="p2")
                        p2v = p2[:].rearrange("c (h w) -> c h w", h=14)
                        cnt = 0
                        for kh in range(3):
                            for kw in range(3):
                                base = (kh + 14 * hf) * H2P + kw
                                rhs = s1[:, :, base: base + 420]
                                nc.tensor.matmul(
                                    p2[:], w2t[:, kh * 3 + kw, ko], rhs,
                                    start=(cnt == 0), stop=False,
                                    perf_mode=DRPM)
                                cnt += 1
                        for limb in (hi, lo):
                            qf = limb[:].rearrange("c a b h w -> c a b (h w)")
                            rhs = qf[:, 1, 1,
                                     14 * hf * QW: 14 * hf * QW + 420]
                            cnt += 1
                            nc.tensor.matmul(
                                p2[:], wsct[:, ko, :], rhs,
                                start=False, stop=(cnt == 11))
                        nc.scalar.activation(
                            ou[:, ko, 14 * hf: 14 * hf + 14, :],
                            p2v[:, :, 0:OW], AF.Sign)
                nc.sync.dma_start(
                    y_d[i].rearrange("(ko m) h w -> m ko h w", ko=2), ou[:])

            def whole_pass():
                prep_w()
                prev = None
                for i in range(img):
                    hi, lo = load(i)
                    s1 = conv1(i, hi, lo)
                    if prev is not None:
                        conv2_out(*prev)
                    prev = (i, s1, hi, lo)
                conv2_out(*prev)

            if repeat == 1:
                whole_pass()
            else:
                with tc.For_i(0, repeat, 1):
                    whole_pass()

    nc.compile()
    return nc


def _get_nc(repeat=1):
    if repeat not in _CACHE:
        _CACHE[repeat] = _build(repeat=repeat)
    return _CACHE[repeat]


def prep_weights(w1, w2, w_sc):
    """Host-side lhsT layout prep (pure transposition, no math)."""
    w1 = np.asarray(w1, dtype=np.float32)
    w2 = np.asarray(w2, dtype=np.float32)
    w_sc = np.asarray(w_sc, dtype=np.float32)
    # [c, kh*kw, ko, m] from (K=ko*128+m, c, kh, kw)
    w1t = np.ascontiguousarray(
        w1.transpose(1, 2, 3, 0).reshape(P, 9, 2, P))
    # [cp, kh*kw, ko, ct, m] from (K, C=ct*128+cp, kh, kw)
    w2t = np.ascontiguousarray(
        w2.reshape(2, P, 2, P, 3, 3)           # ko m ct cp kh kw
        .transpose(3, 4, 5, 0, 2, 1)           # cp kh kw ko ct m
        .reshape(P, 9, 2, 2, P))
    wsct = np.ascontiguousarray(
        w_sc[:, :, 0, 0].transpose(1, 0).reshape(P, 2, P))
    return w1t, w2t, wsct


def kernel(x, w1, w2, w_sc):
    from concourse import bass_utils

    x = np.ascontiguousarray(np.asarray(x, dtype=np.float32))
    w1t, w2t, wsct = prep_weights(w1, w2, w_sc)

    nc = _get_nc()
    in_maps = [
        {"x": x[c * IMG:(c + 1) * IMG], "w1": w1t, "w2": w2t, "wsc": wsct}
        for c in range(N_CORES)
    ]
    res = bass_utils.run_bass_kernel_spmd(
        nc, in_maps, core_ids=list(range(N_CORES)))
    y = np.concatenate([res.results[c]["y"] for c in range(N_CORES)], axis=0)
    return y
